# revision 1
# baseline (speedup 1.0000x reference)
"""Trainium2 Bass kernel for CrossAttention.

  y = softmax((x@Wq) @ (ctx@Wk)^T / sqrt(D)) @ (ctx@Wv) @ Wo + bo

Shapes: x [16, 4096, 1024], context [16, 77, 768], H=8 heads, D=64.
Sharding: pure data-parallel over batch B — each of the 8 cores gets 2
batches; no collectives.

Per-core device program (all matmuls bf16, fp32 PSUM accumulation),
software-pipelined over 16 macro-tiles of 512 tokens:

  iter i emission:  [xt DMA prefetch]  [qproj(i): 32 MMs + 4 ACT copies]
                    [back(i-1) phase A: attnV MMs + DVE normalize,
                     with scores(i) MMs + exp ACTs interleaved]
                    [back(i-1) phase B: PE transposes + ACT copies]
                    [back(i-1) phase C: outproj MMs + DVE bias-add + y DMA]

The phase separation keeps the PE queue free of short dependency chains
(attnV -> normalize -> transpose -> copy -> outproj): by the time the PE
reaches phase B of iter i-1, the DVE normalizes finished during phase A;
ACT copies finish during phase B/C. Scores/exp are spread through phase A
so the 2-buffer scores PSUM pool never stalls the PE on ACT exp latency.
y is written bf16 (host casts to f32) to halve the output DMA traffic.
"""

import os

import numpy as np
import ml_dtypes

import bass_rust
import concourse.bass as bass
import concourse.mybir as mybir
import concourse.tile as _tile
from concourse.bass_utils import run_bass_kernel_spmd
from concourse.masks import make_identity
from concourse.vector_clock import ScopedClock

# ---------------------------------------------------------------------------
# Workaround: this walrus build rejects >1 sem-wait on one SP CTRL
# instruction ("Too many sync wait commands").  Split the Tile tail-drain
# waits across multiple Drain instructions (one wait each).
_MAXW = 1


def _split_drain_and_barrier(self, tick_clock, wait_clock):
    nc = self.nc
    drain_inst = nc.sync.drain()
    wait_clock.add_sem_waits(
        drain_inst.ins, ScopedClock({None: tick_clock.global_clock})
    )
    si = drain_inst.ins.sync_info
    if si is not None and len(si.on_wait) > _MAXW:
        waits = list(si.on_wait)
        upd = list(si.on_update)
        drain_inst.ins.sync_info = bass_rust.SyncInfo(
            on_wait=waits[:_MAXW], on_update=upd
        )
        for i in range(_MAXW, len(waits), _MAXW):
            extra = nc.sync.drain()
            extra.ins.sync_info = bass_rust.SyncInfo(
                on_wait=waits[i : i + _MAXW], on_update=[]
            )
    nc.all_engine_barrier()
    assert self.sems is not None
    popped = nc._tile_sem_poison_stack.pop()
    assert popped is self._sem_poison
    nc.clear_and_free_semaphores(list(self.sems.allocated().values()))
    nc.all_engine_barrier()


_tile.TileContext._drain_and_barrier = _split_drain_and_barrier

_ws_counter = [0]


def _split_excess_waits(nc, maxw=_MAXW):
    """Walrus here accepts only `maxw` sem-waits per instruction; move the
    excess onto preceding same-engine NoOps (identical blocking semantics)."""
    for fn in nc.m.functions:
        for bb in fn.blocks:
            new = []
            for inst in bb.instructions:
                si = inst.sync_info
                if si is not None and len(si.on_wait) > maxw:
                    waits = list(si.on_wait)
                    upd = list(si.on_update)
                    extra, keep = waits[:-maxw], waits[-maxw:]
                    for i in range(0, len(extra), maxw):
                        nop = mybir.InstNoOp(
                            name=f"waitsplit-{_ws_counter[0]}", ins=[], outs=[]
                        )
                        _ws_counter[0] += 1
                        nop.engine = inst.engine
                        nop.sync_info = bass_rust.SyncInfo(
                            on_wait=extra[i : i + maxw], on_update=[]
                        )
                        new.append(nop)
                    inst.sync_info = bass_rust.SyncInfo(
                        on_wait=keep, on_update=upd
                    )
                new.append(inst)
            bb.instructions = new

# ---------------------------------------------------------------------------
# Problem constants (hardcoded per contract)
B, N, M = 16, 4096, 77
Q_DIM, C_DIM = 1024, 768
H, D = 8, 64
INNER = H * D  # 512
N_CORES = 8
B_LOC = B // N_CORES  # 2 batches per core

P = 128
KQ = Q_DIM // P  # 8 feature chunks of x
KC = C_DIM // P  # 6 feature chunks of context
IC = INNER // P  # 4 inner chunks
TQ = 512  # tokens per macro-tile
NT = N // TQ  # 8 macro-tiles per batch
TC = TQ // P  # 4 token chunks of 128 inside a macro-tile
NIT = B_LOC * NT  # 16 flat pipeline iterations

BF16 = mybir.dt.bfloat16
F32 = mybir.dt.float32

LAST_RESULTS = None  # BassKernelResults of the most recent run (for test.py)


def _build_program():
    nc = bass.Bass()
    xT = nc.dram_tensor("xT", [B_LOC, Q_DIM, N], BF16, kind="ExternalInput")
    ctxT = nc.dram_tensor("ctxT", [B_LOC, C_DIM, M], BF16, kind="ExternalInput")
    wq = nc.dram_tensor("wq", [Q_DIM, INNER], BF16, kind="ExternalInput")
    wk = nc.dram_tensor("wk", [C_DIM, INNER], BF16, kind="ExternalInput")
    wv = nc.dram_tensor("wv", [C_DIM, INNER], BF16, kind="ExternalInput")
    wo = nc.dram_tensor("wo", [INNER, Q_DIM], BF16, kind="ExternalInput")
    bo = nc.dram_tensor("bo", [P, Q_DIM], BF16, kind="ExternalInput")
    y = nc.dram_tensor("y", [B_LOC * N, Q_DIM], BF16, kind="ExternalOutput")

    with _tile.TileContext(nc) as tc:
        with (
            tc.tile_pool(name="const", bufs=1) as const,
            tc.tile_pool(name="kv", bufs=2) as kvp,
            tc.tile_pool(name="kt", bufs=8) as ktp,
            tc.tile_pool(name="xin", bufs=3) as xp,
            tc.tile_pool(name="qt", bufs=8) as qp,
            tc.tile_pool(name="st", bufs=18) as sp,
            tc.tile_pool(name="ob", bufs=6) as op_,
            tc.tile_pool(name="otb", bufs=6) as otp,
            tc.tile_pool(name="rcb", bufs=6) as rcp,
            tc.tile_pool(name="yo", bufs=6) as yp,
            tc.tile_pool(name="ps_qy", bufs=2, space="PSUM") as ps_qy,
            tc.tile_pool(name="ps_s", bufs=2, space="PSUM") as ps_s,
            tc.tile_pool(name="ps_o", bufs=2, space="PSUM") as ps_o,
            tc.tile_pool(name="ps_t", bufs=2, space="PSUM") as ps_t,
        ):
            # ---- constants / weights ----
            wq_sb = const.tile([P, KQ, INNER], BF16)
            nc.sync.dma_start(out=wq_sb[:], in_=wq.rearrange("(k p) i -> p k i", p=P))
            wk_sb = const.tile([P, KC, INNER], BF16)
            nc.sync.dma_start(out=wk_sb[:], in_=wk.rearrange("(k p) i -> p k i", p=P))
            wv_sb = const.tile([P, KC, INNER], BF16)
            nc.sync.dma_start(out=wv_sb[:], in_=wv.rearrange("(k p) i -> p k i", p=P))
            wo_sb = const.tile([P, IC, Q_DIM], BF16)
            nc.sync.dma_start(out=wo_sb[:], in_=wo.rearrange("(c p) o -> p c o", p=P))
            bo_bc = const.tile([P, Q_DIM], BF16)
            nc.sync.dma_start(out=bo_bc[:], in_=bo[:, :])
            ident = const.tile([P, P], BF16)
            make_identity(nc, ident[:])

            # ---- per-batch K^T / V(+ones) setup, both batches up front ----
            kts = {}  # (b, c) -> [128, M] tile; head 2c rows 0:64, 2c+1 rows 64:128
            vas = {}  # b -> [M, H, 65]
            for b in range(B_LOC):
                ctx_sb = kvp.tile([P, KC, M], BF16, tag="ctx")
                nc.sync.dma_start(
                    out=ctx_sb[:], in_=ctxT[b].rearrange("(k p) m -> p k m", p=P)
                )
                for c in range(IC):
                    pk = ps_s.tile([P, M], F32, tag="s")
                    for f in range(KC):
                        nc.tensor.matmul(
                            pk[:],
                            lhsT=wk_sb[:, f, c * P : (c + 1) * P],
                            rhs=ctx_sb[:, f, :],
                            start=(f == 0),
                            stop=(f == KC - 1),
                        )
                    kt = ktp.tile([P, M], BF16, tag="kt")
                    nc.vector.tensor_copy(kt[:], pk[:])
                    kts[(b, c)] = kt
                va = kvp.tile([M, H, 65], BF16, tag="va")
                pv = ps_s.tile([M, INNER], F32, tag="s")
                for f in range(KC):
                    nc.tensor.matmul(
                        pv[:],
                        lhsT=ctx_sb[:, f, :],
                        rhs=wv_sb[:, f, :],
                        start=(f == 0),
                        stop=(f == KC - 1),
                    )
                nc.vector.tensor_copy(
                    va[:, :, 0:64], pv.rearrange("p (h d) -> p h d", h=H)
                )
                nc.vector.memset(va[:, :, 64:65], 1.0)
                vas[b] = va

            # ---- software-pipelined macro-tile loop ----
            def load_xt(i):
                b, t = divmod(i, NT)
                t0 = t * TQ
                xt = xp.tile([P, KQ, TQ], BF16, tag="x")
                nc.sync.dma_start(
                    out=xt[:],
                    in_=xT[b].rearrange("(k p) t -> p k t", p=P)[
                        :, :, t0 : t0 + TQ
                    ],
                )
                return xt

            def emit_qproj(xt):
                qts = []
                for c in range(IC):
                    pq = ps_qy.tile([P, TQ], F32, tag="qy")
                    for k in range(KQ):
                        nc.tensor.matmul(
                            pq[:],
                            lhsT=wq_sb[:, k, c * P : (c + 1) * P],
                            rhs=xt[:, k, :],
                            start=(k == 0),
                            stop=(k == KQ - 1),
                        )
                    qt = qp.tile([P, TQ], BF16, tag="qt")
                    nc.scalar.copy(qt[:], pq[:])
                    qts.append(qt)
                return qts

            def emit_scores1(b, qts, h):
                """Scores + exp for head h; returns the sT tile."""
                c, hh = h // 2, h % 2
                rows = slice(hh * 64, (hh + 1) * 64)
                ps = ps_s.tile([M, TQ], F32, tag="s")
                nc.tensor.matmul(
                    ps[:],
                    lhsT=kts[(b, c)][rows, :],
                    rhs=qts[c][rows, :],
                    start=True,
                    stop=True,
                )
                st = sp.tile([M, TQ], BF16, tag="st")
                nc.scalar.activation(
                    st[:], ps[:], mybir.ActivationFunctionType.Exp
                )
                return st

            state = {}  # i -> dict(xt, qts, sts)
            pending = None  # (i, sts, b)

            def emit_back(i, sts, b, front_cb):
                """Back-half for iter i; front_cb(j) is called at interleave
                points j=0..3 to emit the next iteration's scores/exp."""
                # ---- phase A: attnV + normalize (+ interleaved scores) ----
                o_sbs = []
                for tcc in range(TC):
                    tok = slice(tcc * P, (tcc + 1) * P)
                    o_sb = op_.tile([P, INNER], BF16, tag="o")
                    o_v = o_sb.rearrange("p (h d) -> p h d", d=64)
                    rec = rcp.tile([P, H, 1], F32, tag="rec")
                    for g in range(2):
                        po = ps_o.tile([P, 4 * 65], F32, tag="o")
                        for j in range(4):
                            h = g * 4 + j
                            nc.tensor.matmul(
                                po[:, j * 65 : (j + 1) * 65],
                                lhsT=sts[h][:, tok],
                                rhs=vas[b][:, h, :],
                                start=True,
                                stop=True,
                            )
                        pov = po.rearrange("p (h x) -> p h x", x=65)
                        nc.vector.reciprocal(
                            rec[:, g * 4 : (g + 1) * 4, :], pov[:, :, 64:65]
                        )
                        nc.vector.tensor_tensor(
                            out=o_v[:, g * 4 : (g + 1) * 4, :],
                            in0=pov[:, :, 0:64],
                            in1=rec.rearrange("p h x -> p (h x)")[
                                :, g * 4 : (g + 1) * 4
                            ].broadcast_to([P, 4, 64]),
                            op=mybir.AluOpType.mult,
                        )
                    o_sbs.append(o_sb)
                    front_cb(tcc)
                # ---- phase B: transposes + copies (copies split ACT/DVE),
                # staggered with phase C per tcc ----
                ots = []

                def emit_transp(tcc):
                    ot = otp.tile([P, IC, P], BF16, tag="ot")
                    for icc in range(IC):
                        pt = ps_t.tile([P, P], BF16, tag="t")
                        nc.tensor.transpose(
                            pt[:], o_sbs[tcc][:, icc * P : (icc + 1) * P], ident[:]
                        )
                        if icc % 2 == 0:
                            nc.scalar.copy(ot[:, icc, :], pt[:])
                        else:
                            nc.vector.tensor_copy(ot[:, icc, :], pt[:])
                    ots.append(ot)
                    front_cb(TC + tcc)

                # ---- phase C: outproj + bias + store ----
                bq, t = divmod(i, NT)
                emit_transp(0)
                emit_transp(1)
                for tcc in range(TC):
                    if tcc + 2 < TC:
                        emit_transp(tcc + 2)
                    py0 = ps_qy.tile([P, 512], F32, tag="qy")
                    py1 = ps_qy.tile([P, 512], F32, tag="qy")
                    pys = [py0, py1]
                    for icc in range(IC):
                        for half in range(2):
                            nc.tensor.matmul(
                                pys[half][:],
                                lhsT=ots[tcc][:, icc, :],
                                rhs=wo_sb[:, icc, half * 512 : (half + 1) * 512],
                                start=(icc == 0),
                                stop=(icc == IC - 1),
                            )
                    ysb = yp.tile([P, Q_DIM], BF16, tag="y")
                    for half in range(2):
                        col = slice(half * 512, (half + 1) * 512)
                        nc.vector.tensor_tensor(
                            out=ysb[:, col],
                            in0=pys[half][:],
                            in1=bo_bc[:, col],
                            op=mybir.AluOpType.add,
                        )
                    row0 = bq * N + t * TQ + tcc * P
                    nc.sync.dma_start(out=y[row0 : row0 + P, :], in_=ysb[:])

            # prefetch first two x tiles
            xts = {0: load_xt(0), 1: load_xt(1)}

            for i in range(NIT):
                b = i // NT
                if i + 2 < NIT:
                    xts[i + 2] = load_xt(i + 2)
                qts = emit_qproj(xts.pop(i))

                if pending is None:
                    # iter 0: no back-half yet; emit scores directly
                    sts = [emit_scores1(b, qts, h) for h in range(H)]
                    pending = (i, sts, b)
                    continue

                sts_next = []

                def front_cb(j, _b=b, _q=qts, _acc=sts_next):
                    if j < H:
                        _acc.append(emit_scores1(_b, _q, j))

                pi, psts, pb = pending
                emit_back(pi, psts, pb, front_cb)
                pending = (i, sts_next, b)

            pi, psts, pb = pending
            emit_back(pi, psts, pb, lambda j: None)

    _split_excess_waits(nc)
    return nc


def prep_in_maps(x, context, Wq, Wk, Wv, Wo, bo):
    bf = ml_dtypes.bfloat16
    # host-side prep: bf16 cast + pre-transpose so contraction dims are
    # contiguous on device partitions
    xT = np.ascontiguousarray(
        np.asarray(x, dtype=np.float32).transpose(0, 2, 1)
    ).astype(bf)
    ctxT = np.ascontiguousarray(
        np.asarray(context, dtype=np.float32).transpose(0, 2, 1)
    ).astype(bf)
    scale = np.float32(1.0 / np.sqrt(D))
    wq_h = (np.asarray(Wq, dtype=np.float32) * scale).astype(bf)
    wk_h = np.asarray(Wk, dtype=np.float32).astype(bf)
    wv_h = np.asarray(Wv, dtype=np.float32).astype(bf)
    wo_h = np.asarray(Wo, dtype=np.float32).astype(bf)
    bo_h = np.ascontiguousarray(
        np.broadcast_to(
            np.asarray(bo, dtype=np.float32).reshape(1, Q_DIM), (P, Q_DIM)
        )
    ).astype(bf)
    in_maps = []
    for c in range(N_CORES):
        in_maps.append(
            {
                "xT": xT[c * B_LOC : (c + 1) * B_LOC],
                "ctxT": ctxT[c * B_LOC : (c + 1) * B_LOC],
                "wq": wq_h,
                "wk": wk_h,
                "wv": wv_h,
                "wo": wo_h,
                "bo": bo_h,
            }
        )
    return in_maps


def kernel(x, context, Wq, Wk, Wv, Wo, bo):
    global LAST_RESULTS
    in_maps = prep_in_maps(x, context, Wq, Wk, Wv, Wo, bo)
    nc = _build_program()
    trace = bool(int(os.environ.get("BASS_KERNEL_TRACE", "0")))
    res = run_bass_kernel_spmd(
        nc, in_maps, core_ids=list(range(N_CORES)), trace=trace
    )
    LAST_RESULTS = res
    out = np.empty((B, N, Q_DIM), dtype=np.float32)
    for c in range(N_CORES):
        out[c * B_LOC : (c + 1) * B_LOC] = (
            res.results[c]["y"].astype(np.float32).reshape(B_LOC, N, Q_DIM)
        )
    return out

